# revision 4
# baseline (speedup 1.0000x reference)
"""Adaptive embedding lookup (4 vocab buckets, per-bucket projection) on 8 TRN2 cores.

Strategy: token-parallel SPMD, one dma_gather per vocab bucket.

Host side: tokens are bucketed by vocab range, sorted by table row, and dealt
to the 8 cores as balanced *contiguous* chunks of the sorted order, so each
core's rows for a bucket span a narrow window of the table. Each core gets its
own bf16 copy of exactly that window uploaded as an input, which keeps gather
indices within int16 range regardless of vocab size. Projections are
pre-transposed, EMB_SCALE-folded, and packed into two bf16 SBUF images.

Device side (per core):
  - one SWDGE dma_gather per bucket, transpose=True: fetches all of the
    bucket's rows in a single ~1us instruction AND lands them embed-dim-major
    (contract dim on partitions), so no PE transposes are needed at all.
    Buckets with rows < 256B gather a 256B element spanning several
    consecutive rows; the matmul simply only reads the first row's partitions.
  - bf16 matmuls against the packed projections, accumulating in PSUM
  - PSUM -> SBUF bf16 casts split across Vector/Scalar engines
  - bf16 output tiles DMA'd to DRAM from the Sync engine's HWDGE queue
A short burst of dummy matmuls at graph start ramps the PE p-state clock
(0.65 -> 1.2 -> 2.4 GHz after 3us busy) while the first gathers land.
Host inverse-permutes the 8 bf16 shards into the full f32 output.
"""
import sys

import numpy as np

if "/opt/trn_rl_repo" not in sys.path:
    sys.path.insert(0, "/opt/trn_rl_repo")

import ml_dtypes  # noqa: E402
from concourse import bacc, bass, mybir, tile  # noqa: E402
from concourse.bass_utils import run_bass_kernel_spmd  # noqa: E402

N_CORES = 8
P = 128
CUTS = [0, 20000, 40000, 200000, 267735]
N_BUCKETS = 4
D_PROJ = 1024
EMB_SCALE = float(D_PROJ) ** 0.5
D_EMB = [1024, 256, 64, 16]
ELEM = [1024, 256, 128, 128]  # gather element size (bf16 elems), >=256B each
RPE = [1, 1, 2, 8]  # consecutive table rows packed per window row
IDX_SPAN = 32000  # max rows one gather segment may span (int16 headroom)
SEG_CAP = 896  # max indices per dma_gather (HW ucode fails somewhere in (896, 1024])

F32 = mybir.dt.float32
BF16 = mybir.dt.bfloat16
I16 = mybir.dt.int16
BF16NP = ml_dtypes.bfloat16

# compute/gather order: b2 first (most tiles, smallest proj dependency),
# b0 last (needs the 2MB ptB image, which streams in behind ptA)
BUCKET_ORDER = [2, 3, 1, 0]


def _cdiv(a, b):
    return -(-a // b)


def _build_graph(plan):
    """plan: dict with segs (ordered list of (b, s)), N/W per seg, idx col
    offsets, out row offsets, ptA/ptB column layout."""
    nc = bacc.Bacc(None, target_bir_lowering=False, debug=False)

    C = plan["idx_cols"]
    idx_p = nc.declare_dram_parameter("idx", [P, C], I16, isOutput=False)
    w_p = {}
    for (b, s) in plan["segs"]:
        w_p[(b, s)] = nc.declare_dram_parameter(
            f"w{b}_{s}", [plan["W"][(b, s)], ELEM[b]], BF16, isOutput=False
        )
    ptA_p = nc.declare_dram_parameter("ptA", [P, plan["ptA_cols"]], BF16, isOutput=False)
    ptB_p = nc.declare_dram_parameter("ptB", [P, 8 * 1024], BF16, isOutput=False)
    R = plan["out_rows"]
    out_p = nc.declare_dram_parameter("out", [R, D_PROJ], BF16, isOutput=True)

    with tile.TileContext(nc) as tc:
        with (
            tc.tile_pool(name="persist", bufs=1) as pp,
            tc.tile_pool(name="outs", bufs=6) as op,
            tc.tile_pool(name="ps_mm", bufs=2, space="PSUM") as ps_mm,
            tc.tile_pool(name="ps_warm", bufs=1, space="PSUM") as ps_warm,
        ):
            # idx load first on the sync HWDGE queue (fast fixed overhead);
            # everything gathers off it
            idx_sb = pp.tile([P, C], I16)
            nc.sync.dma_start(out=idx_sb[:], in_=idx_p[:])

            # PE warmup: ramp the p-state clock while gathers land
            warm = pp.tile([P, 512], BF16, tag="warm")
            nc.vector.memset(warm[:], 0)
            wps = ps_warm.tile([P, 512], F32, tag="warm_ps")
            for _ in range(10):
                nc.tensor.matmul(wps[:], warm[:, :P], warm[:], start=True, stop=True)

            # packed projections: ptA = [b2 | b3 | b1] chunks, ptB = b0's 8
            ptA_sb = pp.tile([P, plan["ptA_cols"]], BF16, tag="ptA")
            nc.scalar.dma_start(out=ptA_sb[:], in_=ptA_p[:])
            ptB_sb = pp.tile([P, 8 * 1024], BF16, tag="ptB")
            nc.scalar.dma_start(out=ptB_sb[:], in_=ptB_p[:])

            # one transposing gather per bucket segment
            g_sb = {}
            for (b, s) in plan["segs"]:
                kc = ELEM[b] // P
                N = plan["N"][(b, s)]
                g = pp.tile([P, kc, N], BF16, tag=f"g{b}_{s}")
                o = plan["idx_off"][(b, s)]
                nc.gpsimd.dma_gather(
                    g[:, :, :],
                    w_p[(b, s)][:, :],
                    idx_sb[:, o : o + N // 16],
                    N,
                    N,
                    ELEM[b],
                    transpose=True,
                )
                g_sb[(b, s)] = g

            for (b, s) in plan["segs"]:
                N = plan["N"][(b, s)]
                g = g_sb[(b, s)]
                d = D_EMB[b]
                kc = _cdiv(d, P)
                pt_sb = ptB_sb if b == 0 else ptA_sb
                pt_off = plan["pt_off"][b]
                r0_seg = plan["row_off"][(b, s)]
                for j in range(N // P):
                    mm0 = ps_mm.tile([P, 512], F32, tag="mm0")
                    mm1 = ps_mm.tile([P, 512], F32, tag="mm1")
                    mms = [mm0, mm1]
                    for k in range(kc):
                        cw = min(P, d - k * P)
                        lhsT = g[0:cw, k, j * P : (j + 1) * P]
                        for h in range(2):
                            nc.tensor.matmul(
                                mms[h][:, :],
                                lhsT,
                                pt_sb[0:cw, pt_off + k * 1024 + h * 512 : pt_off + k * 1024 + (h + 1) * 512],
                                start=(k == 0),
                                stop=(k == kc - 1),
                            )
                    out_sb = op.tile([P, D_PROJ], BF16, tag="o")
                    nc.vector.tensor_copy(out=out_sb[:, 0:512], in_=mm0[:, :])
                    nc.scalar.activation(
                        out=out_sb[:, 512:1024],
                        in_=mm1[:, :],
                        func=mybir.ActivationFunctionType.Copy,
                    )
                    r0 = r0_seg + j * P
                    nc.sync.dma_start(out=out_p[r0 : r0 + P, :], in_=out_sb[:, :])

    nc.compile()
    return nc


def _make_windows(table_bf, start, W, rpe):
    """Rows [start, start+W) of the rpe-rows-per-element packed view of
    table_bf, zero-padded past the table end."""
    v, d = table_bf.shape
    out = np.zeros((W, rpe * d), dtype=BF16NP)
    take = min(W, v - start)
    if rpe == 1:
        out[:take] = table_bf[start : start + take]
        return out
    pad = np.zeros((rpe - 1, d), dtype=BF16NP)
    tbp = np.concatenate([table_bf[start : start + take + rpe - 1], pad])[
        : take + rpe - 1
    ]
    if tbp.shape[0] < take + rpe - 1:
        tbp = np.concatenate(
            [tbp, np.zeros((take + rpe - 1 - tbp.shape[0], d), dtype=BF16NP)]
        )
    sw = np.lib.stride_tricks.sliding_window_view(tbp, rpe, axis=0)  # [take, d, rpe]
    out[:take] = np.ascontiguousarray(sw.transpose(0, 2, 1)).reshape(take, rpe * d)
    return out


def kernel(inp, emb0, emb1, emb2, emb3, proj0, proj1, proj2, proj3):
    embs = [np.asarray(e, dtype=np.float32) for e in (emb0, emb1, emb2, emb3)]
    projs = [proj0, proj1, proj2, proj3]
    v_emb = [e.shape[0] for e in embs]
    embs_bf = [e.astype(BF16NP) for e in embs]

    inp = np.asarray(inp)
    orig_shape = inp.shape
    flat = inp.reshape(-1).astype(np.int64)

    bucket = np.digitize(flat, CUTS[1:-1])  # 0..3
    local = flat - np.asarray(CUTS, dtype=np.int64)[bucket]

    # per bucket: sort by row, deal balanced contiguous chunks to cores,
    # then greedy-split each core's chunk into segments spanning <= IDX_SPAN
    # rows (normally exactly one)
    core_segs = {b: [[] for _ in range(N_CORES)] for b in range(N_BUCKETS)}
    for b in range(N_BUCKETS):
        pos = np.nonzero(bucket == b)[0]
        loc = np.clip(local[pos], 0, v_emb[b] - 1)
        srt = np.argsort(loc, kind="stable")
        pos, loc = pos[srt], loc[srt]
        n = len(pos)
        base, rem = divmod(n, N_CORES)
        ofs = 0
        for c in range(N_CORES):
            cnt = base + (1 if c < rem else 0)
            lc, pc = loc[ofs : ofs + cnt], pos[ofs : ofs + cnt]
            ofs += cnt
            segs = []
            i = 0
            while i < len(lc):
                start = int(lc[i])
                jend = min(
                    int(np.searchsorted(lc, start + IDX_SPAN, side="left")),
                    i + SEG_CAP,
                )
                segs.append((start, lc[i:jend], pc[i:jend]))
                i = jend
            if not segs:
                segs = [(0, lc[:0], pc[:0])]
            core_segs[b][c] = segs

    # uniform SPMD shapes: per bucket, G segments; per segment, N idx slots
    # (multiple of 128, padded with idx 0) and W window rows (max span)
    plan = {"segs": [], "N": {}, "W": {}, "idx_off": {}, "row_off": {}}
    for b in BUCKET_ORDER:
        G = max(len(core_segs[b][c]) for c in range(N_CORES))
        for c in range(N_CORES):
            while len(core_segs[b][c]) < G:
                core_segs[b][c].append((0, np.zeros(0, np.int64), np.zeros(0, np.int64)))
        for s in range(G):
            plan["segs"].append((b, s))
            maxn = max(len(core_segs[b][c][s][1]) for c in range(N_CORES))
            plan["N"][(b, s)] = max(P, _cdiv(maxn, P) * P)
            maxw = 1
            for c in range(N_CORES):
                st, lc, _ = core_segs[b][c][s]
                if len(lc):
                    maxw = max(maxw, int(lc[-1]) - st + 1)
            plan["W"][(b, s)] = maxw

    co = 0
    ro = 0
    for (b, s) in plan["segs"]:
        plan["idx_off"][(b, s)] = co
        plan["row_off"][(b, s)] = ro
        co += plan["N"][(b, s)] // 16
        ro += plan["N"][(b, s)]
    plan["idx_cols"] = co
    plan["out_rows"] = ro

    # packed projection images: ptA = [b2 | b3 | b1 chunks], ptB = b0 chunks
    pt_scaled = [
        (np.asarray(projs[b], dtype=np.float32).T * EMB_SCALE) for b in range(N_BUCKETS)
    ]  # [d_b, 1024]
    plan["pt_off"] = {2: 0, 3: 1024, 1: 2048, 0: 0}
    plan["ptA_cols"] = 4096
    ptA = np.zeros((P, 4096), dtype=np.float32)
    ptA[0:64, 0:1024] = pt_scaled[2]
    ptA[0:16, 1024:2048] = pt_scaled[3]
    ptA[:, 2048:3072] = pt_scaled[1][0:128]
    ptA[:, 3072:4096] = pt_scaled[1][128:256]
    ptB = np.zeros((P, 8 * 1024), dtype=np.float32)
    for k in range(8):
        ptB[:, k * 1024 : (k + 1) * 1024] = pt_scaled[0][k * P : (k + 1) * P]
    ptA = ptA.astype(BF16NP)
    ptB = ptB.astype(BF16NP)

    nc = _build_graph(plan)

    # per-core inputs: idx image + per-segment table windows
    in_maps = []
    for c in range(N_CORES):
        im = {"ptA": ptA, "ptB": ptB}
        idx_img = np.zeros((P, plan["idx_cols"]), dtype=np.int16)
        for (b, s) in plan["segs"]:
            st, lc, _ = core_segs[b][c][s]
            N = plan["N"][(b, s)]
            rel = np.zeros(N, dtype=np.int16)
            rel[: len(lc)] = (lc - st).astype(np.int16)
            o = plan["idx_off"][(b, s)]
            wrapped = rel.reshape(N // 16, 16).T  # [16, N/16]
            idx_img[:, o : o + N // 16] = np.tile(wrapped, (8, 1))
            im[f"w{b}_{s}"] = _make_windows(
                embs_bf[b], st, plan["W"][(b, s)], RPE[b]
            )
        im["idx"] = idx_img
        in_maps.append(im)

    res = run_bass_kernel_spmd(nc, in_maps, core_ids=list(range(N_CORES)))

    out_full = np.zeros((flat.shape[0], D_PROJ), dtype=np.float32)
    for c in range(N_CORES):
        shard = res.results[c]["out"]
        for (b, s) in plan["segs"]:
            _, lc, pc = core_segs[b][c][s]
            if len(pc):
                r0 = plan["row_off"][(b, s)]
                out_full[pc] = shard[r0 : r0 + len(pc)].astype(np.float32)
    return out_full.reshape(*orig_shape, D_PROJ)
